# revision 31
# baseline (speedup 1.0000x reference)
"""Trainium2 Bass kernel for nn_KLDLoss_18769007083961.

Math reformulation (validated vs reference):
  For each image b, prototype a with class c(a), define over pixels p:
    em_a[p] = exp(d_a[p]) * (label[p] == c(a))          # masked weights
    Z_a     = sum_p em_a[p]
    G[a,j]  = sum_p em_a[p] * d_j[p]                    # needed for same-class (a,j)
    A[a,j]  = G[a,j] / Z_a
  Symmetric KL for a same-class pair (i,j) (log-partition terms cancel):
    kld = 0.5 * (A[j,j] - A[j,i] + A[i,i] - A[i,j])
  loss = mean over valid pairs (class count >= 2) of exp(-kld).

Only same-class (pixel, prototype) products ever contribute, so the host
sorts each image's pixels by label into fixed per-class column blocks
(51 columns of 128 pixels per class; slack slots padded with -1e4 so
exp underflows to exactly 0).  Each pixel slot carries the 9 values the
math needs: its own class's 8 prototype distances (class-major order)
plus a 1.0 for the Z row.  The class structure is thus fully encoded in
the layout: the device kernel is just DMA -> exp -> per-column matmuls.

Device kernel (one image per NeuronCore, 8 cores):
  dist[w, q, i*9+n]: column j = FI*w + i holds 128 sorted pixels, n in
  0..7 = own-class protos, n = 8 = 1.0.  Per window: ACT computes
  em = exp(d[:, :, 0:8]) in fp16, then FI fp16 matmuls (lhsT = d-slice
  [128, 9], rhs = em-slice [128, 8]) accumulate into the class's PSUM
  block g[9c : 9c+9, 0:8]: rows 0..7 = G[a, j], row 8 = Z.  The host
  does the tiny 120-pair combination.
"""

import sys
from contextlib import ExitStack

import numpy as np

sys.path.insert(0, "/opt/trn_rl_repo")

import concourse.bass as bass
import concourse.tile as tile
from concourse import mybir
from concourse.bass_utils import run_bass_kernel_spmd
from concourse.tile import add_dep_helper

B = 8
C = 10
NPROT = 80
NS = 9           # 8 own-class protos + ones slot per pixel
P = 65536
Q = 128          # partitions (pixels per column)
CB_MIN = 49      # minimum column budget per class (49*128 = 6272)
F32 = mybir.dt.float32
F16 = mybir.dt.float16

_NC_CACHE = {}


def _chunks(ncol):
    """Column ranges per window: ascending sizes so the first exp lands
    early while later chunks keep the matmul stream gap-free."""
    a = ncol // 10
    b = ncol // 5
    c = 3 * ncol // 10
    return [(0, a), (a, a + b), (a + b, a + b + c), (a + b + c, ncol)]


def build_nc(cb):
    ncol = cb * C
    d_in = None
    nc = bass.Bass()
    d_in = nc.dram_tensor("dist", [Q, ncol * NS], F16, kind="ExternalInput")
    g_out = nc.dram_tensor("g", [NS, 8 * C], F32, kind="ExternalOutput")

    with ExitStack() as ctx:
        # Dummy 1-element exp emitted before the TileContext lands in the
        # entry block, so walrus places the ACT exp-table load there and it
        # overlaps the fixed engine-init prologue instead of gating exp(0).
        scratch = ctx.enter_context(nc.sbuf_tensor([1, 2], F32))
        nc.scalar.activation(
            scratch.ap()[0:1, 0:1],
            scratch.ap()[0:1, 1:2],
            mybir.ActivationFunctionType.Exp,
        )

        tc = ctx.enter_context(tile.TileContext(nc))
        sbuf = ctx.enter_context(tc.tile_pool(name="sbuf", bufs=1))
        psum = ctx.enter_context(tc.tile_pool(name="psum", bufs=1, space="PSUM"))
        dpool = empool = sbuf

        g_ps = psum.tile([NS, 8 * C], F32)

        bounds = _chunks(ncol)
        d_tiles = []
        for w, (c0, c1) in enumerate(bounds):
            d_t = dpool.tile([Q, (c1 - c0) * NS], F16, tag=f"d{w}", bufs=1,
                             name=f"d_t{w}")
            eng = nc.sync if w % 2 == 0 else nc.scalar
            eng.dma_start(out=d_t, in_=d_in[:, c0 * NS : c1 * NS])
            d_tiles.append(d_t)

        prev_exp = None
        for w, (c0, c1) in enumerate(bounds):
            d_t = d_tiles[w]
            fi = c1 - c0

            em_t = empool.tile([Q, fi * NS], F16, tag=f"em{w}", bufs=1,
                               name=f"em_t{w}")
            d_v = d_t.rearrange("p (i n) -> p i n", n=NS)
            em_v = em_t.rearrange("p (i n) -> p i n", n=NS)
            i_exp = nc.scalar.activation(
                em_v[:, :, 0:8], d_v[:, :, 0:8], mybir.ActivationFunctionType.Exp
            )
            if prev_exp is not None:
                add_dep_helper(i_exp.ins, prev_exp.ins, sync=False)
            prev_exp = i_exp

            for i in range(fi):
                j = c0 + i
                cls = min(j // cb, C - 1)
                nc.tensor.matmul(
                    g_ps[:, 8 * cls : 8 * (cls + 1)],
                    d_t[:, i * NS : (i + 1) * NS],
                    em_t[:, i * NS : i * NS + 8],
                    start=(j == cb * cls),
                    stop=(j == cb * (cls + 1) - 1),
                    skip_group_check=True,
                )

        # Copy finished class blocks out of PSUM in two halves so the
        # first copy overlaps the remaining matmul stream.
        g_sb = sbuf.tile([NS, 8 * C], F32)
        nc.vector.tensor_copy(g_sb[:, : 8 * 5], g_ps[:, : 8 * 5])
        nc.vector.tensor_copy(g_sb[:, 8 * 5 :], g_ps[:, 8 * 5 :])
        nc.sync.dma_start(out=g_out[:, :], in_=g_sb)

    # The input DMAs and the ACT exp-table load have no dependencies; hoist
    # them ahead of the TileContext entry barriers so the data transfer and
    # table load overlap the fixed preamble instead of following it.
    for fn in nc.m.functions:
        if not fn.blocks:
            continue
        hoist = []
        for blk in fn.blocks:
            for ins in list(blk.instructions):
                if type(ins).__name__ in (
                    "InstDMACopy",
                    "InstActTableLoad",
                ) and not (ins.sync_info and ins.sync_info.on_wait):
                    hoist.append(ins)
                    blk.instructions.remove(ins)
        insts0 = fn.blocks[0].instructions
        first_drain = next(
            (k for k, ins in enumerate(insts0)
             if type(ins).__name__ == "InstDrain"),
            len(insts0),
        )
        for k, ins in enumerate(hoist):
            insts0.insert(first_drain + k, ins)

    # Hardware instruction structs hold only one sync wait.  Move any excess
    # waits onto single-wait InstDrains injected just before the instruction
    # on the same engine queue (the union of waits still precedes execution).
    import copy as _copy

    drain_tmpl = {}
    for fn in nc.m.functions:
        for blk in fn.blocks:
            for ins in blk.instructions:
                if type(ins).__name__ == "InstDrain" and ins.engine is not None:
                    drain_tmpl.setdefault(ins.engine, ins)

    seq = [0]

    def _drain_clone(engine, wait):
        tmpl = drain_tmpl[engine]
        d2 = _copy.deepcopy(tmpl)
        seq[0] += 1
        d2.name = f"waitsplit-{seq[0]}"
        d2.sync_info = type(tmpl.sync_info)(on_wait=[wait], on_update=[])
        return d2

    for fn in nc.m.functions:
        for blk in fn.blocks:
            insts = blk.instructions
            idx = 0
            while idx < len(insts):
                ins = insts[idx]
                si = ins.sync_info
                if si and len(si.on_wait) > 1 and ins.engine in drain_tmpl:
                    waits = list(si.on_wait)
                    si.on_wait = waits[-1:]
                    for k, wt in enumerate(waits[:-1]):
                        insts.insert(idx + k, _drain_clone(ins.engine, wt))
                    idx += len(waits) - 1
                idx += 1

    return nc


def _get_nc(cb):
    if cb not in _NC_CACHE:
        _NC_CACHE[cb] = build_nc(cb)
    return _NC_CACHE[cb]


def run_device(dist8, cb, trace=False):
    """dist8: [8, Q, ncol*9] fp16 sorted/padded layout."""
    nc = _get_nc(cb)
    in_maps = [{"dist": dist8[b]} for b in range(B)]
    return run_bass_kernel_spmd(nc, in_maps, list(range(B)), trace=trace)


def kernel(
    prototype_distances,
    target_labels,
    proto_class,
    pair_i,
    pair_j,
    pair_cls,
    _trace=False,
    _results_out=None,
):
    dist = np.asarray(prototype_distances, dtype=np.float32).reshape(B, NPROT, P)
    labels = np.asarray(target_labels).reshape(B, P).astype(np.int64)
    proto_class = np.asarray(proto_class, dtype=np.int64)
    pair_i = np.asarray(pair_i, dtype=np.int64)
    pair_j = np.asarray(pair_j, dtype=np.int64)
    pair_cls = np.asarray(pair_cls, dtype=np.int64)

    # Permute prototypes class-major: slot n holds a prototype of class n//8.
    perm = np.empty(NPROT, dtype=np.int64)
    for c in range(C):
        protos = np.nonzero(proto_class == c)[0]
        assert len(protos) == 8, "expect 8 prototypes per class"
        perm[8 * c : 8 * (c + 1)] = protos
    inv = np.empty(NPROT, dtype=np.int64)
    inv[perm] = np.arange(NPROT)

    # Sort pixels by label into fixed per-class column blocks and pack the
    # 9 needed values per pixel slot; pad slack slots with -1e4 (exp -> 0).
    lab = labels - 1                       # [B, P], -1 = ignore
    cnts = np.stack([(lab == c).sum(axis=1) for c in range(C)], axis=1)
    cb = max(CB_MIN, int(-(-cnts.max() // Q)))
    ncol = cb * C
    dist_p = np.full((B, ncol, Q, NS), -1.0e4, dtype=np.float16)
    dist_p[..., 8] = 1.0
    dperm = dist[:, perm, :]               # [B, 80, P] class-major
    for b in range(B):
        for c in range(C):
            idx = np.nonzero(lab[b] == c)[0]
            cnt = len(idx)
            rho = np.arange(cnt)
            cols = cb * c + rho // Q
            rows = rho % Q
            vals = dperm[b, 8 * c : 8 * (c + 1), :][:, idx]  # [8, cnt]
            dist_p[b, cols, rows, 0:8] = vals.T.astype(np.float16)
    dist8 = np.ascontiguousarray(
        dist_p.transpose(0, 2, 1, 3).reshape(B, Q, ncol * NS)
    )

    br = run_device(dist8, cb, trace=_trace)
    if _results_out is not None:
        _results_out.append(br)

    total_vals = np.float64(0.0)
    total_valid = 0
    for b in range(B):
        out = br.results[b]["g"]  # [9, 80]
        A = np.zeros((NPROT, NPROT), dtype=np.float64)
        for c in range(C):
            blk = out[:, 8 * c : 8 * (c + 1)].astype(np.float64)  # [9, 8]
            Z = blk[8]                                           # [8]
            Gc = blk[0:8]                                        # Gc[j, a] = sum em_a d_j
            with np.errstate(divide="ignore", invalid="ignore"):
                Ac = np.where(Z[None, :] != 0.0, Gc / Z[None, :], 0.0)
            A[8 * c : 8 * (c + 1), 8 * c : 8 * (c + 1)] = Ac
        cnt = cnts[b]
        ii = inv[pair_i]
        jj = inv[pair_j]
        kld = 0.5 * (A[jj, jj] - A[jj, ii] + A[ii, ii] - A[ii, jj])
        valid = cnt[pair_cls] >= 2
        total_vals += np.exp(-kld[valid]).sum()
        total_valid += int(valid.sum())

    if total_valid > 0:
        res = np.float32(total_vals / max(total_valid, 1))
    else:
        res = np.float32(0.0)
    return res


if __name__ == "__main__":
    rng = np.random.default_rng(0)
    d = rng.standard_normal((B, NPROT, 256, 256), dtype=np.float32)
    l = rng.integers(0, 11, (B, 256, 256))
    pc = (np.arange(NPROT) % 40) // 4
    pairs = []
    for s in range(2):
        for c in range(C):
            base = s * 40 + c * 4
            for a in range(4):
                for b2 in range(a + 1, 4):
                    pairs.append((base + a, base + b2, c))
    pairs = np.asarray(pairs, np.int32)
    print(kernel(d, l, pc, pairs[:, 0], pairs[:, 1], pairs[:, 2]))
